# revision 49
# baseline (speedup 1.0000x reference)
"""Trainium2 kernel for nn_DifferentiableSuperpixelTokenizer (segment mean of
linearly-projected pixel features).

segment_mean(concat(img, xy) @ W + b) == (segsum(feat6) @ [W; b]) / clamp(counts, 1)
with feat6 = (r, g, b, x, y, 1): six per-pixel features, reduced into 196 bins
per batch, then one tiny [196, 6] @ [6, 768] projection (linearity of the
projection makes this exact).

Sharding: data-parallel over batch -> 8 NeuronCores, one batch element each;
host slices inputs / stacks outputs. Per core, 392 chunks of 128 pixels:
DVE (and every 5th chunk ScalarE) builds a [128, 196] fp16 one-hot; TensorE
accumulates feat_chunk.T @ onehot into PSUM with 4-way col-tiling; the counts
reciprocal is broadcast across feature rows via a tiny selector matmul and
applied before the projection. All PSUM tiles are padded to full banks (the allocator otherwise packs sub-bank tiles at offsets
where a matmul output straddles banks).

Includes workarounds for this stack: walrus accepts only ONE sem-wait per
instruction (extra waits are peeled onto same-engine NoOps; same-engine
self-waits dropped for DVE/Pool only -- they complete in order); the Tile tail
barrier is replaced by drain-waits emitted on Pool, which owns the sem clear.
"""
import numpy as np
from contextlib import ExitStack

import bass_rust
import concourse.mybir as mybir
import concourse.tile as tile
from concourse.tile import ScopedClock

MAX_INST_WAITS = 1

import os

# Dropping same-engine waits (relying on in-order completion) corrupts the
# tail's cnt/rec/ob chain on hardware; keep them (peeled into NoOps) unless
# explicitly asked to drop.
if os.environ.get("BASS_DROP_SELF_WAITS"):
    _SELF_DROP_ENGINES = {
        mybir.EngineType.DVE: "DVE",
        mybir.EngineType.Pool: "Pool",
    }
else:
    _SELF_DROP_ENGINES = {}


def _split_waits(ins):
    """Return leftover waits to emit as preceding nops; mutates ins."""
    si = getattr(ins, "sync_info", None)
    if si is None:
        return []
    waits = list(si.on_wait)
    if not waits:
        return []
    self_name = _SELF_DROP_ENGINES.get(ins.engine)
    if self_name is not None:
        kept = [w for w in waits if w.ant_name.rsplit("_", 1)[0] != self_name]
    else:
        kept = waits
    head = kept[:-MAX_INST_WAITS] if len(kept) > MAX_INST_WAITS else []
    rest = kept[len(head):]
    if len(waits) != len(rest) or head:
        ins.sync_info = bass_rust.SyncInfo(
            on_wait=rest, on_update=list(si.on_update)
        )
    return head


_orig_commit = tile.TileContext._commit_instruction


def _patched_commit(self, inst, lazy_reg_writes=True):
    head = _split_waits(inst)
    for i in range(0, len(head), MAX_INST_WAITS):
        nop = mybir.InstNoOp(
            name=self.nc.get_next_instruction_name(),
            sync_info=mybir.SyncInfo(
                on_wait=head[i : i + MAX_INST_WAITS], on_update=[]
            ),
            bass_nofuse=True,
            engine=inst.engine,
        )
        _orig_commit(self, nop, lazy_reg_writes=False)
    return _orig_commit(self, inst, lazy_reg_writes)


def _patched_drain_and_barrier(self, tick_clock, wait_clock):
    # Emit the global-clock drain waits on Pool (one wait per Drain), since
    # Pool is also the engine performing the final sem clear: program order
    # on Pool replaces the expensive all-engine EVSEM barrier.
    nc = self.nc
    drain_inst = nc.gpsimd.drain()
    wait_clock.add_sem_waits(
        drain_inst.ins, ScopedClock({None: tick_clock.global_clock})
    )
    si = drain_inst.ins.sync_info
    waits = list(si.on_wait) if si is not None else []
    if len(waits) > 1:
        drain_inst.ins.sync_info = bass_rust.SyncInfo(
            on_wait=waits[:1], on_update=[]
        )
        for w in waits[1:]:
            d2 = nc.gpsimd.drain()
            d2.ins.sync_info = bass_rust.SyncInfo(on_wait=[w], on_update=[])

    assert self.sems is not None
    popped = nc._tile_sem_poison_stack.pop()
    assert popped is self._sem_poison
    nc.clear_and_free_semaphores(list(self.sems.allocated().values()))


tile.TileContext._drain_and_barrier = _patched_drain_and_barrier
tile.TileContext._commit_instruction = _patched_commit


import concourse.bass as bass  # noqa: E402

P, T, S, E, F = 128, 392, 196, 768, 6
N = P * T
H = Wimg = 224
B = 8

import ml_dtypes

FP16 = mybir.dt.bfloat16  # 16-bit compute dtype (bf16: packed DVE modes + PE native)
NP16 = ml_dtypes.bfloat16
F32 = mybir.dt.float32


def _make_coords():
    x = np.arange(Wimg, dtype=np.float32) / np.float32(Wimg - 1)
    y = np.arange(H, dtype=np.float32) / np.float32(H - 1)
    xg = np.broadcast_to(x[None, :], (H, Wimg))
    yg = np.broadcast_to(y[:, None], (H, Wimg))
    return np.stack([xg.ravel(), yg.ravel()])  # [2, N] (x, y)


# erf'(0) = 2/sqrt(pi) as produced in the 16-bit ACT one-hot; its group's
# combine rows divide the scale back out so all groups sum unit one-hots.
ERFDX0 = float(np.asarray(2.0 / np.sqrt(np.pi)).astype(NP16))
ACT_GROUP = 3  # PSUM column-group reserved for ACT-built one-hots
CW = 770  # combw6 columns: 768 projected + counts col + pad


def _prep_core_inputs(img, segments, W, b):
    """Per-core in_maps: bf16 feat/seg + shared combw6.

    combw6[32j+i, e] = gscale(j) * [W; b][i, e] and col 768 carries the
    counts selector, so out_raw = acc.T @ combw6 yields both the projected
    segment sums and the per-segment counts in one matmul.
    """
    coords = _make_coords().reshape(2, P, T)
    ones = np.ones((1, P, T), np.float32)
    w6 = np.concatenate([W, b[None, :]], 0).astype(np.float32)  # [6, 768]
    combw6 = np.zeros((P, CW), np.float32)
    for j in range(4):
        g = 1.0 / ERFDX0 if j == ACT_GROUP else 1.0
        combw6[32 * j : 32 * j + F, 0:E] = w6 * g
        combw6[32 * j + F - 1, E] = g
    combw6 = np.ascontiguousarray(combw6.astype(NP16))
    maps = []
    for bi in range(B):
        imgb = img[bi].reshape(3, P, T).astype(np.float32)
        feat6 = np.concatenate([imgb, coords, ones], 0)  # [6, P, T]
        feat_host = np.ascontiguousarray(
            feat6.transpose(1, 2, 0).reshape(P, T * F)
        ).astype(NP16)
        seg_host = np.ascontiguousarray(segments[bi].reshape(P, T)).astype(NP16)
        maps.append({"feat": feat_host, "seg": seg_host, "combw6": combw6})
    return maps


def _build_program(col_tile=True, oh_bufs=8, pattern=("dve",), race_detect=True):
    nc = bass.Bass("TRN2", debug=False, detect_race_conditions=race_detect)
    feat = nc.dram_tensor("feat", [P, T * F], FP16, kind="ExternalInput")
    seg = nc.dram_tensor("seg", [P, T], FP16, kind="ExternalInput")
    combw6 = nc.dram_tensor("combw6", [P, CW], FP16, kind="ExternalInput")
    out = nc.dram_tensor("out", [S, E], FP16, kind="ExternalOutput")

    ngroups = 4 if col_tile else 1
    use_act = "act" in pattern
    # ACT one-hots are erf'(4d) = (2/sqrt(pi)) * onehot; they accumulate in
    # PSUM column-group ACT_GROUP whose combw6 rows divide the scale back out.
    act_group = ACT_GROUP if ngroups > 1 else None
    rot_groups = [j for j in range(ngroups) if j != act_group]

    # iota row replicated across partitions (compare operand)
    iota_np = np.ascontiguousarray(
        np.broadcast_to(np.arange(S).astype(NP16), (P, S))
    )
    iota_c = nc.inline_tensor(iota_np, name="iota_const")

    with tile.TileContext(nc) as tc, ExitStack() as ctx:
        sb = ctx.enter_context(tc.tile_pool(name="sb", bufs=1))
        ohp = ctx.enter_context(tc.tile_pool(name="oh", bufs=oh_bufs))
        pp = ctx.enter_context(tc.tile_pool(name="psum", bufs=1, space="PSUM"))

        # The data DMA queue is FIFO: issue seg+iota (what V needs first),
        # then feat in quarters so early matmul chunks land early; the
        # tail-only combw6 goes last, via gpsimd. iota2/nseg4 for ACT are
        # produced on scalar in parallel.
        # Tiles are created in the layout order that measured fastest (SBUF
        # bank placement is allocation-order dependent). DMA completion sems
        # lag ~2us each and serialize per queue, so the critical transfers
        # go first-in-line on separate engine queues: scalar: iota, gpsimd:
        # seg, sync: feat quarters; tail-only combw6 rides gpsimd last.
        seg_sb = sb.tile([P, T], FP16)
        iota_sb = sb.tile([P, S], FP16)
        # separate copy for ACT to avoid SBUF bank contention with DVE reads
        iota2_sb = sb.tile([P, S], FP16)
        feat_sb = sb.tile([P, T * F], FP16)
        combw6_sb = sb.tile([P, CW], FP16)
        nc.scalar.dma_start(out=iota_sb[:], in_=iota_c.ap()[:, :])
        nc.gpsimd.dma_start(out=seg_sb[:], in_=seg.ap()[:, :])
        nc.gpsimd.dma_start(out=iota2_sb[:], in_=iota_c.ap()[:, :])
        for q in range(4):
            qs = q * (T * F // 4)
            qe = (q + 1) * (T * F // 4)
            nc.sync.dma_start(out=feat_sb[:, qs:qe], in_=feat.ap()[:, qs:qe])
        nc.gpsimd.dma_start(out=combw6_sb[:], in_=combw6.ap()[:, :])

        # fp32 copy of seg for the is_equal scalar operand
        seg32_sb = sb.tile([P, T], F32)
        nc.vector.tensor_copy(seg32_sb[:], seg_sb[:])
        if use_act:
            nseg4_sb = sb.tile([P, T], F32)
            nc.scalar.activation(
                nseg4_sb[:],
                seg_sb[:],
                mybir.ActivationFunctionType.Copy,
                bias=0.0,
                scale=-4.0,
            )

        # chunk -> engine, chunk -> PSUM column-group (act has its own group).
        # The last chunks go to the faster V queue so the final matmuls are
        # not head-of-line blocked behind a slow ACT one-hot.
        engs = [pattern[t % len(pattern)] for t in range(T)]
        groups = []
        rr = 0
        for eng in engs:
            if eng == "act" and act_group is not None:
                groups.append(act_group)
            else:
                groups.append(rot_groups[rr % len(rot_groups)])
                rr += 1
        first_t = {}
        last_t = {}
        for t, j in enumerate(groups):
            first_t.setdefault(j, t)
            last_t[j] = t

        # phase 1: segment sums accumulated in PSUM
        acc = pp.tile([P, 512], F32)  # full bank so PSUM tiles stay bank-aligned
        # zero the rows the col-tiled matmuls never touch (comb does 0*garbage
        # otherwise, and PSUM garbage can be NaN)
        nc.vector.memset(acc[:], 0.0)
        for t in range(T):
            j = groups[t]
            eng = engs[t]
            oh = ohp.tile([P, S], FP16, tag=f"oh_{eng}")
            if eng == "dve":
                nc.vector.tensor_scalar(
                    out=oh[:],
                    in0=iota_sb[:],
                    scalar1=seg32_sb[:, t : t + 1],
                    scalar2=None,
                    op0=mybir.AluOpType.is_equal,
                )
            elif eng == "gpsimd":
                nc.gpsimd.tensor_scalar(
                    out=oh[:],
                    in0=iota_sb[:],
                    scalar1=seg32_sb[:, t : t + 1],
                    scalar2=None,
                    op0=mybir.AluOpType.is_equal,
                )
            elif eng == "act":
                # erf'(4*(iota-seg)) = (2/sqrt(pi)) * exact one-hot in fp16
                nc.scalar.activation(
                    oh[:],
                    iota2_sb[:],
                    mybir.ActivationFunctionType.Derivative_Erf,
                    bias=nseg4_sb[:, t : t + 1],
                    scale=4.0,
                )
            else:
                raise ValueError(eng)
            nc.tensor.matmul(
                acc[32 * j : 32 * j + F, 0:S],
                lhsT=feat_sb[:, F * t : F * (t + 1)],
                rhs=oh[:],
                start=(t == first_t[j]),
                stop=(t == last_t[j]),
                tile_position=(0, 32 * j) if col_tile else None,
                skip_group_check=True,
            )

        # tail: out_raw[s, 0:768] = acc.T @ combw6 gives the projected segment
        # sums, col 768 the counts (segments now sit on PSUM partitions, so
        # the reciprocal is a per-partition FD=1 op); then scale each row.
        acc_sb = sb.tile([P, S], FP16)
        with nc.allow_low_precision(reason="bf16 tail; tolerance is 2e-2"):
            nc.vector.tensor_copy(acc_sb[:], acc[:, 0:S])
        for lo, hi in ((0, P), (P, S)):
            m = hi - lo
            otile = pp.tile([P, 1024], F32, tag=f"ot{lo}")
            nc.tensor.matmul(
                otile[:m, 512:CW],
                lhsT=acc_sb[:, lo:hi],
                rhs=combw6_sb[:, 512:CW],
                start=True,
                stop=True,
            )
            nc.tensor.matmul(
                otile[:m, 0:512],
                lhsT=acc_sb[:, lo:hi],
                rhs=combw6_sb[:, 0:512],
                start=True,
                stop=True,
            )
            cnt_sb = sb.tile([P, 1], F32, tag=f"cnt{lo}")
            nc.vector.tensor_scalar(
                out=cnt_sb[:m],
                in0=otile[:m, E : E + 1],
                scalar1=1.0,
                scalar2=None,
                op0=mybir.AluOpType.max,
            )
            rec_sb = sb.tile([P, 1], F32, tag=f"rec{lo}")
            nc.vector.reciprocal(rec_sb[:m], cnt_sb[:m])
            ob = sb.tile([P, E], FP16, tag=f"ob{lo}")
            with nc.allow_low_precision(reason="bf16 out; tolerance is 2e-2"):
                nc.vector.tensor_scalar(
                    out=ob[:m],
                    in0=otile[:m, 0:E],
                    scalar1=rec_sb[:m, 0:1],
                    scalar2=None,
                    op0=mybir.AluOpType.mult,
                )
            nc.sync.dma_start(out=out.ap()[lo:hi, :], in_=ob[:m, :])

    return nc




_PROGRAM_CACHE = {}
_PATTERN = ("dve", "dve", "act")


def run(inputs, trace=False, pattern=_PATTERN, oh_bufs=12, **bkw):
    from concourse.bass_utils import run_bass_kernel_spmd

    img = np.asarray(inputs["img"]).astype(np.float32)
    segments = np.asarray(inputs["segments"])
    W = np.asarray(inputs["W"]).astype(np.float32)
    b = np.asarray(inputs["b"]).astype(np.float32)

    in_maps = _prep_core_inputs(img, segments, W, b)
    nc = _build_program(pattern=pattern, oh_bufs=oh_bufs)
    res = run_bass_kernel_spmd(nc, in_maps, list(range(B)), trace=trace, **bkw)
    out = np.stack(
        [np.asarray(res.results[i]["out"]) for i in range(B)]
    ).astype(np.float32)
    return out, res


def kernel(**inputs) -> np.ndarray:
    out, _ = run(inputs)
    return out



# revision 50
# speedup vs baseline: 1.1888x; 1.1888x over previous
"""Trainium2 kernel for nn_DifferentiableSuperpixelTokenizer (segment mean of
linearly-projected pixel features).

segment_mean(concat(img, xy) @ W + b) == (segsum(feat6) @ [W; b]) / clamp(counts, 1)
with feat6 = (r, g, b, x, y, 1): six per-pixel features, reduced into 196 bins
per batch, then one tiny [196, 6] @ [6, 768] projection (linearity of the
projection makes this exact).

Sharding: data-parallel over batch -> 8 NeuronCores, one batch element each;
host slices inputs / stacks outputs. Per core, 392 chunks of 128 pixels:
DVE (and every 5th chunk ScalarE) builds a [128, 196] fp16 one-hot; TensorE
accumulates feat_chunk.T @ onehot into PSUM with 4-way col-tiling; the counts
reciprocal is broadcast across feature rows via a tiny selector matmul and
applied before the projection. All PSUM tiles are padded to full banks (the allocator otherwise packs sub-bank tiles at offsets
where a matmul output straddles banks).

Includes workarounds for this stack: walrus accepts only ONE sem-wait per
instruction (extra waits are peeled onto same-engine NoOps; same-engine
self-waits dropped for DVE/Pool only -- they complete in order); the Tile tail
barrier is replaced by drain-waits emitted on Pool, which owns the sem clear.
"""
import numpy as np
from contextlib import ExitStack

import bass_rust
import concourse.mybir as mybir
import concourse.tile as tile
from concourse.tile import ScopedClock

MAX_INST_WAITS = 1

import os

# Dropping same-engine waits (relying on in-order completion) corrupts the
# tail's cnt/rec/ob chain on hardware; keep them (peeled into NoOps) unless
# explicitly asked to drop.
if os.environ.get("BASS_DROP_SELF_WAITS"):
    _SELF_DROP_ENGINES = {
        mybir.EngineType.DVE: "DVE",
        mybir.EngineType.Pool: "Pool",
    }
else:
    _SELF_DROP_ENGINES = {}


def _split_waits(ins):
    """Return leftover waits to emit as preceding nops; mutates ins."""
    si = getattr(ins, "sync_info", None)
    if si is None:
        return []
    waits = list(si.on_wait)
    if not waits:
        return []
    self_name = _SELF_DROP_ENGINES.get(ins.engine)
    if self_name is not None:
        kept = [w for w in waits if w.ant_name.rsplit("_", 1)[0] != self_name]
    else:
        kept = waits
    head = kept[:-MAX_INST_WAITS] if len(kept) > MAX_INST_WAITS else []
    rest = kept[len(head):]
    if len(waits) != len(rest) or head:
        ins.sync_info = bass_rust.SyncInfo(
            on_wait=rest, on_update=list(si.on_update)
        )
    return head


_orig_commit = tile.TileContext._commit_instruction


def _patched_commit(self, inst, lazy_reg_writes=True):
    head = _split_waits(inst)
    for i in range(0, len(head), MAX_INST_WAITS):
        nop = mybir.InstNoOp(
            name=self.nc.get_next_instruction_name(),
            sync_info=mybir.SyncInfo(
                on_wait=head[i : i + MAX_INST_WAITS], on_update=[]
            ),
            bass_nofuse=True,
            engine=inst.engine,
        )
        _orig_commit(self, nop, lazy_reg_writes=False)
    return _orig_commit(self, inst, lazy_reg_writes)


def _patched_drain_and_barrier(self, tick_clock, wait_clock):
    # Emit the global-clock drain waits on Pool (one wait per Drain), since
    # Pool is also the engine performing the final sem clear: program order
    # on Pool replaces the expensive all-engine EVSEM barrier.
    nc = self.nc
    drain_inst = nc.gpsimd.drain()
    wait_clock.add_sem_waits(
        drain_inst.ins, ScopedClock({None: tick_clock.global_clock})
    )
    si = drain_inst.ins.sync_info
    waits = list(si.on_wait) if si is not None else []
    if len(waits) > 1:
        drain_inst.ins.sync_info = bass_rust.SyncInfo(
            on_wait=waits[:1], on_update=[]
        )
        for w in waits[1:]:
            d2 = nc.gpsimd.drain()
            d2.ins.sync_info = bass_rust.SyncInfo(on_wait=[w], on_update=[])

    assert self.sems is not None
    popped = nc._tile_sem_poison_stack.pop()
    assert popped is self._sem_poison
    nc.clear_and_free_semaphores(list(self.sems.allocated().values()))


tile.TileContext._drain_and_barrier = _patched_drain_and_barrier
tile.TileContext._commit_instruction = _patched_commit


import concourse.bass as bass  # noqa: E402

P, T, S, E, F = 128, 392, 196, 768, 6
N = P * T
H = Wimg = 224
B = 8

import ml_dtypes

FP16 = mybir.dt.bfloat16  # 16-bit compute dtype (bf16: packed DVE modes + PE native)
NP16 = ml_dtypes.bfloat16
F32 = mybir.dt.float32


def _make_coords():
    x = np.arange(Wimg, dtype=np.float32) / np.float32(Wimg - 1)
    y = np.arange(H, dtype=np.float32) / np.float32(H - 1)
    xg = np.broadcast_to(x[None, :], (H, Wimg))
    yg = np.broadcast_to(y[:, None], (H, Wimg))
    return np.stack([xg.ravel(), yg.ravel()])  # [2, N] (x, y)


# erf'(0) = 2/sqrt(pi) as produced in the 16-bit ACT one-hot; its group's
# combine rows divide the scale back out so all groups sum unit one-hots.
ERFDX0 = float(np.asarray(2.0 / np.sqrt(np.pi)).astype(NP16))
ACT_GROUP = 3  # PSUM column-group reserved for ACT-built one-hots
CW = 770  # combw6 columns: 768 projected + counts col + pad


def _prep_core_inputs(img, segments, W, b):
    """Per-core in_maps: bf16 feat/seg + shared combw6.

    combw6[32j+i, e] = gscale(j) * [W; b][i, e] and col 768 carries the
    counts selector, so out_raw = acc.T @ combw6 yields both the projected
    segment sums and the per-segment counts in one matmul.
    """
    coords = _make_coords().reshape(2, P, T)
    ones = np.ones((1, P, T), np.float32)
    w6 = np.concatenate([W, b[None, :]], 0).astype(np.float32)  # [6, 768]
    combw6 = np.zeros((P, CW), np.float32)
    for j in range(4):
        g = 1.0 / ERFDX0 if j == ACT_GROUP else 1.0
        combw6[32 * j : 32 * j + F, 0:E] = w6 * g
        combw6[32 * j + F - 1, E] = g
    combw6 = np.ascontiguousarray(combw6.astype(NP16))
    maps = []
    for bi in range(B):
        imgb = img[bi].reshape(3, P, T).astype(np.float32)
        feat6 = np.concatenate([imgb, coords, ones], 0)  # [6, P, T]
        feat_host = np.ascontiguousarray(
            feat6.transpose(1, 2, 0).reshape(P, T * F)
        ).astype(NP16)
        seg_host = np.ascontiguousarray(segments[bi].reshape(P, T)).astype(NP16)
        maps.append({"feat": feat_host, "seg": seg_host, "combw6": combw6})
    return maps


def _build_program(col_tile=True, oh_bufs=8, pattern=("dve",), race_detect=True):
    nc = bass.Bass("TRN2", debug=False, detect_race_conditions=race_detect)
    feat = nc.dram_tensor("feat", [P, T * F], FP16, kind="ExternalInput")
    seg = nc.dram_tensor("seg", [P, T], FP16, kind="ExternalInput")
    combw6 = nc.dram_tensor("combw6", [P, CW], FP16, kind="ExternalInput")
    out = nc.dram_tensor("out", [S, E], FP16, kind="ExternalOutput")

    ngroups = 4 if col_tile else 1
    use_act = "act" in pattern
    # ACT one-hots are erf'(4d) = (2/sqrt(pi)) * onehot; they accumulate in
    # PSUM column-group ACT_GROUP whose combw6 rows divide the scale back out.
    act_group = ACT_GROUP if ngroups > 1 else None
    rot_groups = [j for j in range(ngroups) if j != act_group]

    # iota row replicated across partitions (compare operand)
    iota_np = np.ascontiguousarray(
        np.broadcast_to(np.arange(S).astype(NP16), (P, S))
    )
    iota_c = nc.inline_tensor(iota_np, name="iota_const")

    with tile.TileContext(nc) as tc, ExitStack() as ctx:
        sb = ctx.enter_context(tc.tile_pool(name="sb", bufs=1))
        ohp = ctx.enter_context(tc.tile_pool(name="oh", bufs=oh_bufs))
        pp = ctx.enter_context(tc.tile_pool(name="psum", bufs=1, space="PSUM"))

        # The data DMA queue is FIFO: issue seg+iota (what V needs first),
        # then feat in quarters so early matmul chunks land early; the
        # tail-only combw6 goes last, via gpsimd. iota2/nseg4 for ACT are
        # produced on scalar in parallel.
        # Tiles are created in the layout order that measured fastest (SBUF
        # bank placement is allocation-order dependent). DMA completion sems
        # lag ~2us each and serialize per queue, so the critical transfers
        # go first-in-line on separate engine queues: scalar: iota, gpsimd:
        # seg, sync: feat quarters; tail-only combw6 rides gpsimd last.
        seg_sb = sb.tile([P, T], FP16)
        iota_sb = sb.tile([P, S], FP16)
        # separate copy for ACT to avoid SBUF bank contention with DVE reads
        iota2_sb = sb.tile([P, S], FP16)
        feat_sb = sb.tile([P, T * F], FP16)
        combw6_sb = sb.tile([P, CW], FP16)
        nc.sync.dma_start(out=seg_sb[:], in_=seg.ap()[:, :])
        nc.sync.dma_start(out=iota_sb[:], in_=iota_c.ap()[:, :])
        nc.scalar.dma_start(out=iota2_sb[:], in_=iota_c.ap()[:, :])
        for q in range(4):
            qs = q * (T * F // 4)
            qe = (q + 1) * (T * F // 4)
            nc.sync.dma_start(out=feat_sb[:, qs:qe], in_=feat.ap()[:, qs:qe])
        nc.gpsimd.dma_start(out=combw6_sb[:], in_=combw6.ap()[:, :])

        # fp32 copy of seg for the is_equal scalar operand
        seg32_sb = sb.tile([P, T], F32)
        nc.vector.tensor_copy(seg32_sb[:], seg_sb[:])
        if use_act:
            nseg4_sb = sb.tile([P, T], F32)
            nc.scalar.activation(
                nseg4_sb[:],
                seg_sb[:],
                mybir.ActivationFunctionType.Copy,
                bias=0.0,
                scale=-4.0,
            )

        # chunk -> engine, chunk -> PSUM column-group (act has its own group).
        # The last chunks go to the faster V queue so the final matmuls are
        # not head-of-line blocked behind a slow ACT one-hot.
        engs = [pattern[t % len(pattern)] for t in range(T)]
        groups = []
        rr = 0
        for eng in engs:
            if eng == "act" and act_group is not None:
                groups.append(act_group)
            else:
                groups.append(rot_groups[rr % len(rot_groups)])
                rr += 1
        first_t = {}
        last_t = {}
        for t, j in enumerate(groups):
            first_t.setdefault(j, t)
            last_t[j] = t

        # phase 1: segment sums accumulated in PSUM
        acc = pp.tile([P, 512], F32)  # full bank so PSUM tiles stay bank-aligned
        # zero the rows the col-tiled matmuls never touch (comb does 0*garbage
        # otherwise, and PSUM garbage can be NaN)
        nc.vector.memset(acc[:], 0.0)
        for t in range(T):
            j = groups[t]
            eng = engs[t]
            oh = ohp.tile([P, S], FP16, tag=f"oh_{eng}")
            if eng == "dve":
                nc.vector.tensor_scalar(
                    out=oh[:],
                    in0=iota_sb[:],
                    scalar1=seg32_sb[:, t : t + 1],
                    scalar2=None,
                    op0=mybir.AluOpType.is_equal,
                )
            elif eng == "gpsimd":
                nc.gpsimd.tensor_scalar(
                    out=oh[:],
                    in0=iota_sb[:],
                    scalar1=seg32_sb[:, t : t + 1],
                    scalar2=None,
                    op0=mybir.AluOpType.is_equal,
                )
            elif eng == "act":
                # erf'(4*(iota-seg)) = (2/sqrt(pi)) * exact one-hot in fp16
                nc.scalar.activation(
                    oh[:],
                    iota2_sb[:],
                    mybir.ActivationFunctionType.Derivative_Erf,
                    bias=nseg4_sb[:, t : t + 1],
                    scale=4.0,
                )
            else:
                raise ValueError(eng)
            nc.tensor.matmul(
                acc[32 * j : 32 * j + F, 0:S],
                lhsT=feat_sb[:, F * t : F * (t + 1)],
                rhs=oh[:],
                start=(t == first_t[j]),
                stop=(t == last_t[j]),
                tile_position=(0, 32 * j) if col_tile else None,
                skip_group_check=True,
            )

        # tail: out_raw[s, 0:768] = acc.T @ combw6 gives the projected segment
        # sums, col 768 the counts (segments now sit on PSUM partitions, so
        # the reciprocal is a per-partition FD=1 op); then scale each row.
        acc_sb = sb.tile([P, S], FP16)
        with nc.allow_low_precision(reason="bf16 tail; tolerance is 2e-2"):
            nc.vector.tensor_copy(acc_sb[:], acc[:, 0:S])
        for lo, hi in ((0, P), (P, S)):
            m = hi - lo
            otile = pp.tile([P, 1024], F32, tag=f"ot{lo}")
            nc.tensor.matmul(
                otile[:m, 512:CW],
                lhsT=acc_sb[:, lo:hi],
                rhs=combw6_sb[:, 512:CW],
                start=True,
                stop=True,
            )
            nc.tensor.matmul(
                otile[:m, 0:512],
                lhsT=acc_sb[:, lo:hi],
                rhs=combw6_sb[:, 0:512],
                start=True,
                stop=True,
            )
            cnt_sb = sb.tile([P, 1], F32, tag=f"cnt{lo}")
            nc.vector.tensor_scalar(
                out=cnt_sb[:m],
                in0=otile[:m, E : E + 1],
                scalar1=1.0,
                scalar2=None,
                op0=mybir.AluOpType.max,
            )
            rec_sb = sb.tile([P, 1], F32, tag=f"rec{lo}")
            nc.vector.reciprocal(rec_sb[:m], cnt_sb[:m])
            ob = sb.tile([P, E], FP16, tag=f"ob{lo}")
            with nc.allow_low_precision(reason="bf16 out; tolerance is 2e-2"):
                nc.vector.tensor_scalar(
                    out=ob[:m],
                    in0=otile[:m, 0:E],
                    scalar1=rec_sb[:m, 0:1],
                    scalar2=None,
                    op0=mybir.AluOpType.mult,
                )
            nc.sync.dma_start(out=out.ap()[lo:hi, :], in_=ob[:m, :])

    return nc




_PROGRAM_CACHE = {}
_PATTERN = ("dve", "dve", "act")


def run(inputs, trace=False, pattern=_PATTERN, oh_bufs=12, **bkw):
    from concourse.bass_utils import run_bass_kernel_spmd

    img = np.asarray(inputs["img"]).astype(np.float32)
    segments = np.asarray(inputs["segments"])
    W = np.asarray(inputs["W"]).astype(np.float32)
    b = np.asarray(inputs["b"]).astype(np.float32)

    in_maps = _prep_core_inputs(img, segments, W, b)
    nc = _build_program(pattern=pattern, oh_bufs=oh_bufs)
    res = run_bass_kernel_spmd(nc, in_maps, list(range(B)), trace=trace, **bkw)
    out = np.stack(
        [np.asarray(res.results[i]["out"]) for i in range(B)]
    ).astype(np.float32)
    return out, res


def kernel(**inputs) -> np.ndarray:
    out, _ = run(inputs)
    return out



# revision 55
# speedup vs baseline: 1.2270x; 1.0321x over previous
"""Trainium2 kernel for nn_DifferentiableSuperpixelTokenizer (segment mean of
linearly-projected pixel features).

segment_mean(concat(img, xy) @ W + b) == (segsum(feat6) @ [W; b]) / clamp(counts, 1)
with feat6 = (r, g, b, x, y, 1): six per-pixel features, reduced into 196 bins
per batch, then one tiny [196, 6] @ [6, 768] projection (linearity of the
projection makes this exact).

Sharding: data-parallel over batch -> 8 NeuronCores, one batch element each;
host slices inputs / stacks outputs. Per core, 392 chunks of 128 pixels:
DVE (and every 5th chunk ScalarE) builds a [128, 196] fp16 one-hot; TensorE
accumulates feat_chunk.T @ onehot into PSUM with 4-way col-tiling; the counts
reciprocal is broadcast across feature rows via a tiny selector matmul and
applied before the projection. All PSUM tiles are padded to full banks (the allocator otherwise packs sub-bank tiles at offsets
where a matmul output straddles banks).

Includes workarounds for this stack: walrus accepts only ONE sem-wait per
instruction (extra waits are peeled onto same-engine NoOps; same-engine
self-waits dropped for DVE/Pool only -- they complete in order); the Tile tail
barrier is replaced by drain-waits emitted on Pool, which owns the sem clear.
"""
import numpy as np
from contextlib import ExitStack

import bass_rust
import concourse.mybir as mybir
import concourse.tile as tile
from concourse.tile import ScopedClock

MAX_INST_WAITS = 1

import os

# Dropping same-engine waits (relying on in-order completion) corrupts the
# tail's cnt/rec/ob chain on hardware; keep them (peeled into NoOps) unless
# explicitly asked to drop.
if os.environ.get("BASS_DROP_SELF_WAITS"):
    _SELF_DROP_ENGINES = {
        mybir.EngineType.DVE: "DVE",
        mybir.EngineType.Pool: "Pool",
    }
else:
    _SELF_DROP_ENGINES = {}


def _split_waits(ins):
    """Return leftover waits to emit as preceding nops; mutates ins."""
    si = getattr(ins, "sync_info", None)
    if si is None:
        return []
    waits = list(si.on_wait)
    if not waits:
        return []
    self_name = _SELF_DROP_ENGINES.get(ins.engine)
    if self_name is not None:
        kept = [w for w in waits if w.ant_name.rsplit("_", 1)[0] != self_name]
    else:
        kept = waits
    head = kept[:-MAX_INST_WAITS] if len(kept) > MAX_INST_WAITS else []
    rest = kept[len(head):]
    if len(waits) != len(rest) or head:
        ins.sync_info = bass_rust.SyncInfo(
            on_wait=rest, on_update=list(si.on_update)
        )
    return head


_orig_commit = tile.TileContext._commit_instruction


def _patched_commit(self, inst, lazy_reg_writes=True):
    head = _split_waits(inst)
    for i in range(0, len(head), MAX_INST_WAITS):
        nop = mybir.InstNoOp(
            name=self.nc.get_next_instruction_name(),
            sync_info=mybir.SyncInfo(
                on_wait=head[i : i + MAX_INST_WAITS], on_update=[]
            ),
            bass_nofuse=True,
            engine=inst.engine,
        )
        _orig_commit(self, nop, lazy_reg_writes=False)
    return _orig_commit(self, inst, lazy_reg_writes)


def _patched_drain_and_barrier(self, tick_clock, wait_clock):
    # Emit the global-clock drain waits on Pool (one wait per Drain), since
    # Pool is also the engine performing the final sem clear: program order
    # on Pool replaces the expensive all-engine EVSEM barrier.
    nc = self.nc
    drain_inst = nc.gpsimd.drain()
    wait_clock.add_sem_waits(
        drain_inst.ins, ScopedClock({None: tick_clock.global_clock})
    )
    si = drain_inst.ins.sync_info
    waits = list(si.on_wait) if si is not None else []
    if len(waits) > 1:
        drain_inst.ins.sync_info = bass_rust.SyncInfo(
            on_wait=waits[:1], on_update=[]
        )
        for w in waits[1:]:
            d2 = nc.gpsimd.drain()
            d2.ins.sync_info = bass_rust.SyncInfo(on_wait=[w], on_update=[])

    assert self.sems is not None
    popped = nc._tile_sem_poison_stack.pop()
    assert popped is self._sem_poison
    nc.clear_and_free_semaphores(list(self.sems.allocated().values()))


tile.TileContext._drain_and_barrier = _patched_drain_and_barrier
tile.TileContext._commit_instruction = _patched_commit


import concourse.bass as bass  # noqa: E402

P, T, S, E, F = 128, 392, 196, 768, 6
N = P * T
H = Wimg = 224
B = 8

import ml_dtypes

FP16 = mybir.dt.bfloat16  # 16-bit compute dtype (bf16: packed DVE modes + PE native)
NP16 = ml_dtypes.bfloat16
F32 = mybir.dt.float32


def _make_coords():
    x = np.arange(Wimg, dtype=np.float32) / np.float32(Wimg - 1)
    y = np.arange(H, dtype=np.float32) / np.float32(H - 1)
    xg = np.broadcast_to(x[None, :], (H, Wimg))
    yg = np.broadcast_to(y[:, None], (H, Wimg))
    return np.stack([xg.ravel(), yg.ravel()])  # [2, N] (x, y)


# erf'(0) = 2/sqrt(pi) as produced in the 16-bit ACT one-hot; its group's
# combine rows divide the scale back out so all groups sum unit one-hots.
ERFDX0 = float(np.asarray(2.0 / np.sqrt(np.pi)).astype(NP16))
ACT_GROUP = 3  # PSUM column-group reserved for ACT-built one-hots
CW = 770  # combw6 columns: 768 projected + counts col + pad


def _prep_core_inputs(img, segments, W, b):
    """Per-core in_maps: bf16 feat/seg + shared combw6.

    combw6[32j+i, e] = gscale(j) * [W; b][i, e] and col 768 carries the
    counts selector, so out_raw = acc.T @ combw6 yields both the projected
    segment sums and the per-segment counts in one matmul.
    """
    coords = _make_coords().reshape(2, P, T)
    ones = np.ones((1, P, T), np.float32)
    w6 = np.concatenate([W, b[None, :]], 0).astype(np.float32)  # [6, 768]
    combw6 = np.zeros((P, CW), np.float32)
    for j in range(4):
        g = 1.0 / ERFDX0 if j == ACT_GROUP else 1.0
        combw6[32 * j : 32 * j + F, 0:E] = w6 * g
        combw6[32 * j + F - 1, E] = g
    combw6 = np.ascontiguousarray(combw6.astype(NP16))
    iota_row = np.broadcast_to(np.arange(S, dtype=np.float32), (P, S))
    maps = []
    for bi in range(B):
        imgb = img[bi].reshape(3, P, T).astype(np.float32)
        feat6 = np.concatenate([imgb, coords, ones], 0)  # [6, P, T]
        feat_host = np.ascontiguousarray(
            feat6.transpose(1, 2, 0).reshape(P, T * F)
        ).astype(NP16)
        # seg and the iota compare-row ride one DMA (one completion latency)
        segio = np.ascontiguousarray(
            np.concatenate([segments[bi].reshape(P, T), iota_row], 1)
        ).astype(NP16)
        maps.append({"feat": feat_host, "segio": segio, "combw6": combw6})
    return maps


def _build_program(col_tile=True, oh_bufs=8, pattern=("dve",), race_detect=True):
    nc = bass.Bass("TRN2", debug=False, detect_race_conditions=race_detect)
    feat = nc.dram_tensor("feat", [P, T * F], FP16, kind="ExternalInput")
    segio = nc.dram_tensor("segio", [P, T + S], FP16, kind="ExternalInput")
    combw6 = nc.dram_tensor("combw6", [P, CW], FP16, kind="ExternalInput")
    out = nc.dram_tensor("out", [S, E], FP16, kind="ExternalOutput")

    ngroups = 4 if col_tile else 1
    use_act = "act" in pattern
    # ACT one-hots are erf'(4d) = (2/sqrt(pi)) * onehot; they accumulate in
    # PSUM column-group ACT_GROUP whose combw6 rows divide the scale back out.
    act_group = ACT_GROUP if ngroups > 1 else None
    rot_groups = [j for j in range(ngroups) if j != act_group]

    # iota row replicated across partitions (compare operand)
    iota_np = np.ascontiguousarray(
        np.broadcast_to(np.arange(S).astype(NP16), (P, S))
    )
    iota_c = nc.inline_tensor(iota_np, name="iota_const")

    with tile.TileContext(nc) as tc, ExitStack() as ctx:
        sb = ctx.enter_context(tc.tile_pool(name="sb", bufs=1))
        ohp = ctx.enter_context(tc.tile_pool(name="oh", bufs=oh_bufs))
        pp = ctx.enter_context(tc.tile_pool(name="psum", bufs=1, space="PSUM"))

        # The data DMA queue is FIFO: issue seg+iota (what V needs first),
        # then feat in quarters so early matmul chunks land early; the
        # tail-only combw6 goes last, via gpsimd. iota2/nseg4 for ACT are
        # produced on scalar in parallel.
        # Tiles are created in the layout order that measured fastest (SBUF
        # bank placement is allocation-order dependent). DMA completion sems
        # lag ~2us each and serialize per queue, so the critical transfers
        # go first-in-line on separate engine queues: scalar: iota, gpsimd:
        # seg, sync: feat quarters; tail-only combw6 rides gpsimd last.
        segio_sb = sb.tile([P, T + S], FP16)
        seg_sb = segio_sb[:, 0:T]
        iota_sb = segio_sb[:, T : T + S]
        # separate copy for ACT to avoid SBUF bank contention with DVE reads
        iota2_sb = sb.tile([P, S], FP16)
        feat_sb = sb.tile([P, T * F], FP16)
        combw6_sb = sb.tile([P, CW], FP16)
        nc.sync.dma_start(out=segio_sb[:], in_=segio.ap()[:, :])
        nc.scalar.dma_start(out=iota2_sb[:], in_=iota_c.ap()[:, :])
        for q in range(4):
            qs = q * (T * F // 4)
            qe = (q + 1) * (T * F // 4)
            nc.sync.dma_start(out=feat_sb[:, qs:qe], in_=feat.ap()[:, qs:qe])
        nc.gpsimd.dma_start(out=combw6_sb[:], in_=combw6.ap()[:, :])

        # fp32 copy of seg for the is_equal scalar operand
        seg32_sb = sb.tile([P, T], F32)
        nc.vector.tensor_copy(seg32_sb[:], seg_sb)
        if use_act:
            nseg4_sb = sb.tile([P, T], F32)
            nc.scalar.activation(
                nseg4_sb[:],
                seg_sb,
                mybir.ActivationFunctionType.Copy,
                bias=0.0,
                scale=-4.0,
            )

        # chunk -> engine, chunk -> PSUM column-group (act has its own group).
        # The last chunks go to the faster V queue so the final matmuls are
        # not head-of-line blocked behind a slow ACT one-hot.
        engs = [pattern[t % len(pattern)] for t in range(T)]
        groups = []
        rr = 0
        for eng in engs:
            if eng == "act" and act_group is not None:
                groups.append(act_group)
            else:
                groups.append(rot_groups[rr % len(rot_groups)])
                rr += 1
        first_t = {}
        last_t = {}
        for t, j in enumerate(groups):
            first_t.setdefault(j, t)
            last_t[j] = t

        # phase 1: segment sums accumulated in PSUM
        acc = pp.tile([P, 512], F32)  # full bank so PSUM tiles stay bank-aligned
        # zero the rows the col-tiled matmuls never touch (comb does 0*garbage
        # otherwise, and PSUM garbage can be NaN)
        nc.vector.memset(acc[:], 0.0)
        for t in range(T):
            j = groups[t]
            eng = engs[t]
            oh = ohp.tile([P, S], FP16, tag=f"oh_{eng}")
            if eng == "dve":
                nc.vector.tensor_scalar(
                    out=oh[:],
                    in0=iota_sb,
                    scalar1=seg32_sb[:, t : t + 1],
                    scalar2=None,
                    op0=mybir.AluOpType.is_equal,
                )
            elif eng == "gpsimd":
                nc.gpsimd.tensor_scalar(
                    out=oh[:],
                    in0=iota_sb,
                    scalar1=seg32_sb[:, t : t + 1],
                    scalar2=None,
                    op0=mybir.AluOpType.is_equal,
                )
            elif eng == "act":
                # erf'(4*(iota-seg)) = (2/sqrt(pi)) * exact one-hot in fp16
                nc.scalar.activation(
                    oh[:],
                    iota2_sb[:],
                    mybir.ActivationFunctionType.Derivative_Erf,
                    bias=nseg4_sb[:, t : t + 1],
                    scale=4.0,
                )
            else:
                raise ValueError(eng)
            nc.tensor.matmul(
                acc[32 * j : 32 * j + F, 0:S],
                lhsT=feat_sb[:, F * t : F * (t + 1)],
                rhs=oh[:],
                start=(t == first_t[j]),
                stop=(t == last_t[j]),
                tile_position=(0, 32 * j) if col_tile else None,
                skip_group_check=True,
            )

        # tail: out_raw[s, 0:768] = acc.T @ combw6 gives the projected segment
        # sums, col 768 the counts (segments now sit on PSUM partitions, so
        # the reciprocal is a per-partition FD=1 op); then scale each row.
        acc_sb = sb.tile([P, S], FP16)
        with nc.allow_low_precision(reason="bf16 tail; tolerance is 2e-2"):
            nc.vector.tensor_copy(acc_sb[:], acc[:, 0:S])
        for lo, hi in ((0, P), (P, S)):
            m = hi - lo
            otile = pp.tile([P, 1024], F32, tag=f"ot{lo}")
            nc.tensor.matmul(
                otile[:m, 512:CW],
                lhsT=acc_sb[:, lo:hi],
                rhs=combw6_sb[:, 512:CW],
                start=True,
                stop=True,
            )
            nc.tensor.matmul(
                otile[:m, 0:512],
                lhsT=acc_sb[:, lo:hi],
                rhs=combw6_sb[:, 0:512],
                start=True,
                stop=True,
            )
            cnt_sb = sb.tile([P, 1], F32, tag=f"cnt{lo}")
            nc.vector.tensor_scalar(
                out=cnt_sb[:m],
                in0=otile[:m, E : E + 1],
                scalar1=1.0,
                scalar2=None,
                op0=mybir.AluOpType.max,
            )
            rec_sb = sb.tile([P, 1], F32, tag=f"rec{lo}")
            nc.vector.reciprocal(rec_sb[:m], cnt_sb[:m])
            # the two row-tiles' scale-by-recip run concurrently: ACT (idle
            # by now) takes the first, V the second
            ob = sb.tile([P, E], FP16, tag=f"ob{lo}")
            if lo == 0:
                nc.scalar.activation(
                    ob[:m],
                    otile[:m, 0:E],
                    mybir.ActivationFunctionType.Copy,
                    scale=rec_sb[:m, 0:1],
                )
            else:
                with nc.allow_low_precision(reason="bf16 out; 2e-2 tol"):
                    nc.vector.tensor_scalar(
                        out=ob[:m],
                        in0=otile[:m, 0:E],
                        scalar1=rec_sb[:m, 0:1],
                        scalar2=None,
                        op0=mybir.AluOpType.mult,
                    )
            nc.sync.dma_start(out=out.ap()[lo:hi, :], in_=ob[:m, :])

    return nc




_PROGRAM_CACHE = {}
_PATTERN = ("dve", "dve", "act")


def run(inputs, trace=False, pattern=_PATTERN, oh_bufs=12, **bkw):
    from concourse.bass_utils import run_bass_kernel_spmd

    img = np.asarray(inputs["img"]).astype(np.float32)
    segments = np.asarray(inputs["segments"])
    W = np.asarray(inputs["W"]).astype(np.float32)
    b = np.asarray(inputs["b"]).astype(np.float32)

    in_maps = _prep_core_inputs(img, segments, W, b)
    nc = _build_program(pattern=pattern, oh_bufs=oh_bufs)
    res = run_bass_kernel_spmd(nc, in_maps, list(range(B)), trace=trace, **bkw)
    out = np.stack(
        [np.asarray(res.results[i]["out"]) for i in range(B)]
    ).astype(np.float32)
    return out, res


def kernel(**inputs) -> np.ndarray:
    out, _ = run(inputs)
    return out



# revision 57
# speedup vs baseline: 1.2403x; 1.0109x over previous
"""Trainium2 kernel for nn_DifferentiableSuperpixelTokenizer (segment mean of
linearly-projected pixel features).

segment_mean(concat(img, xy) @ W + b) == (segsum(feat6) @ [W; b]) / clamp(counts, 1)
with feat6 = (r, g, b, x, y, 1): six per-pixel features, reduced into 196 bins
per batch, then one tiny [196, 6] @ [6, 768] projection (linearity of the
projection makes this exact).

Sharding: data-parallel over batch -> 8 NeuronCores, one batch element each;
host slices inputs / stacks outputs. Per core, 392 chunks of 128 pixels:
DVE (and every 5th chunk ScalarE) builds a [128, 196] fp16 one-hot; TensorE
accumulates feat_chunk.T @ onehot into PSUM with 4-way col-tiling; the counts
reciprocal is broadcast across feature rows via a tiny selector matmul and
applied before the projection. All PSUM tiles are padded to full banks (the allocator otherwise packs sub-bank tiles at offsets
where a matmul output straddles banks).

Includes workarounds for this stack: walrus accepts only ONE sem-wait per
instruction (extra waits are peeled onto same-engine NoOps; same-engine
self-waits dropped for DVE/Pool only -- they complete in order); the Tile tail
barrier is replaced by drain-waits emitted on Pool, which owns the sem clear.
"""
import numpy as np
from contextlib import ExitStack

import bass_rust
import concourse.mybir as mybir
import concourse.tile as tile
from concourse.tile import ScopedClock

MAX_INST_WAITS = 1

import os

# Dropping same-engine waits (relying on in-order completion) corrupts the
# tail's cnt/rec/ob chain on hardware; keep them (peeled into NoOps) unless
# explicitly asked to drop.
if os.environ.get("BASS_DROP_SELF_WAITS"):
    _SELF_DROP_ENGINES = {
        mybir.EngineType.DVE: "DVE",
        mybir.EngineType.Pool: "Pool",
    }
else:
    _SELF_DROP_ENGINES = {}


def _split_waits(ins):
    """Return leftover waits to emit as preceding nops; mutates ins."""
    si = getattr(ins, "sync_info", None)
    if si is None:
        return []
    waits = list(si.on_wait)
    if not waits:
        return []
    self_name = _SELF_DROP_ENGINES.get(ins.engine)
    if self_name is not None:
        kept = [w for w in waits if w.ant_name.rsplit("_", 1)[0] != self_name]
    else:
        kept = waits
    head = kept[:-MAX_INST_WAITS] if len(kept) > MAX_INST_WAITS else []
    rest = kept[len(head):]
    if len(waits) != len(rest) or head:
        ins.sync_info = bass_rust.SyncInfo(
            on_wait=rest, on_update=list(si.on_update)
        )
    return head


_orig_commit = tile.TileContext._commit_instruction


def _patched_commit(self, inst, lazy_reg_writes=True):
    head = _split_waits(inst)
    for i in range(0, len(head), MAX_INST_WAITS):
        nop = mybir.InstNoOp(
            name=self.nc.get_next_instruction_name(),
            sync_info=mybir.SyncInfo(
                on_wait=head[i : i + MAX_INST_WAITS], on_update=[]
            ),
            bass_nofuse=True,
            engine=inst.engine,
        )
        _orig_commit(self, nop, lazy_reg_writes=False)
    return _orig_commit(self, inst, lazy_reg_writes)


def _patched_drain_and_barrier(self, tick_clock, wait_clock):
    # Emit the global-clock drain waits on Pool (one wait per Drain), since
    # Pool is also the engine performing the final sem clear: program order
    # on Pool replaces the expensive all-engine EVSEM barrier.
    nc = self.nc
    drain_inst = nc.gpsimd.drain()
    wait_clock.add_sem_waits(
        drain_inst.ins, ScopedClock({None: tick_clock.global_clock})
    )
    si = drain_inst.ins.sync_info
    waits = list(si.on_wait) if si is not None else []
    if len(waits) > 1:
        drain_inst.ins.sync_info = bass_rust.SyncInfo(
            on_wait=waits[:1], on_update=[]
        )
        for w in waits[1:]:
            d2 = nc.gpsimd.drain()
            d2.ins.sync_info = bass_rust.SyncInfo(on_wait=[w], on_update=[])

    assert self.sems is not None
    popped = nc._tile_sem_poison_stack.pop()
    assert popped is self._sem_poison
    nc.clear_and_free_semaphores(list(self.sems.allocated().values()))


tile.TileContext._drain_and_barrier = _patched_drain_and_barrier
tile.TileContext._commit_instruction = _patched_commit


import concourse.bass as bass  # noqa: E402

P, T, S, E, F = 128, 392, 196, 768, 6
N = P * T
H = Wimg = 224
B = 8

import ml_dtypes

FP16 = mybir.dt.bfloat16  # 16-bit compute dtype (bf16: packed DVE modes + PE native)
NP16 = ml_dtypes.bfloat16
F32 = mybir.dt.float32


def _make_coords():
    x = np.arange(Wimg, dtype=np.float32) / np.float32(Wimg - 1)
    y = np.arange(H, dtype=np.float32) / np.float32(H - 1)
    xg = np.broadcast_to(x[None, :], (H, Wimg))
    yg = np.broadcast_to(y[:, None], (H, Wimg))
    return np.stack([xg.ravel(), yg.ravel()])  # [2, N] (x, y)


# erf'(0) = 2/sqrt(pi) as produced in the 16-bit ACT one-hot; its group's
# combine rows divide the scale back out so all groups sum unit one-hots.
ERFDX0 = float(np.asarray(2.0 / np.sqrt(np.pi)).astype(NP16))
ACT_GROUP = 3  # PSUM column-group reserved for ACT-built one-hots
CW = 770  # combw6 columns: 768 projected + counts col + pad


def _prep_core_inputs(img, segments, W, b):
    """Per-core in_maps: bf16 feat/seg + shared combw6.

    combw6[32j+i, e] = gscale(j) * [W; b][i, e] and col 768 carries the
    counts selector, so out_raw = acc.T @ combw6 yields both the projected
    segment sums and the per-segment counts in one matmul.
    """
    coords = _make_coords().reshape(2, P, T)
    ones = np.ones((1, P, T), np.float32)
    w6 = np.concatenate([W, b[None, :]], 0).astype(np.float32)  # [6, 768]
    combw6 = np.zeros((P, CW), np.float32)
    for j in range(4):
        g = 1.0 / ERFDX0 if j == ACT_GROUP else 1.0
        combw6[32 * j : 32 * j + F, 0:E] = w6 * g
        combw6[32 * j + F - 1, E] = g
    combw6 = np.ascontiguousarray(combw6.astype(NP16))
    iota_row = np.broadcast_to(np.arange(S, dtype=np.float32), (P, S))
    maps = []
    for bi in range(B):
        imgb = img[bi].reshape(3, P, T).astype(np.float32)
        feat6 = np.concatenate([imgb, coords, ones], 0)  # [6, P, T]
        feat_host = np.ascontiguousarray(
            feat6.transpose(1, 2, 0).reshape(P, T * F)
        ).astype(NP16)
        # seg and the iota compare-row ride one DMA (one completion latency)
        segio = np.ascontiguousarray(
            np.concatenate([segments[bi].reshape(P, T), iota_row], 1)
        ).astype(NP16)
        maps.append({"feat": feat_host, "segio": segio, "combw6": combw6})
    return maps


def _build_program(col_tile=True, oh_bufs=8, pattern=("dve",), race_detect=True):
    nc = bass.Bass("TRN2", debug=False, detect_race_conditions=race_detect)
    feat = nc.dram_tensor("feat", [P, T * F], FP16, kind="ExternalInput")
    segio = nc.dram_tensor("segio", [P, T + S], FP16, kind="ExternalInput")
    combw6 = nc.dram_tensor("combw6", [P, CW], FP16, kind="ExternalInput")
    out = nc.dram_tensor("out", [S, E], FP16, kind="ExternalOutput")

    ngroups = 4 if col_tile else 1
    use_act = "act" in pattern
    # ACT one-hots are erf'(4d) = (2/sqrt(pi)) * onehot; they accumulate in
    # PSUM column-group ACT_GROUP whose combw6 rows divide the scale back out.
    act_group = ACT_GROUP if ngroups > 1 else None
    rot_groups = [j for j in range(ngroups) if j != act_group]

    # iota row replicated across partitions (compare operand)
    iota_np = np.ascontiguousarray(
        np.broadcast_to(np.arange(S).astype(NP16), (P, S))
    )
    iota_c = nc.inline_tensor(iota_np, name="iota_const")

    with tile.TileContext(nc) as tc, ExitStack() as ctx:
        sb = ctx.enter_context(tc.tile_pool(name="sb", bufs=1))
        ohp = ctx.enter_context(tc.tile_pool(name="oh", bufs=oh_bufs))
        pp = ctx.enter_context(tc.tile_pool(name="psum", bufs=1, space="PSUM"))

        # The data DMA queue is FIFO: issue seg+iota (what V needs first),
        # then feat in quarters so early matmul chunks land early; the
        # tail-only combw6 goes last, via gpsimd. iota2/nseg4 for ACT are
        # produced on scalar in parallel.
        # Tiles are created in the layout order that measured fastest (SBUF
        # bank placement is allocation-order dependent). DMA completion sems
        # lag ~2us each and serialize per queue, so the critical transfers
        # go first-in-line on separate engine queues: scalar: iota, gpsimd:
        # seg, sync: feat quarters; tail-only combw6 rides gpsimd last.
        segio_sb = sb.tile([P, T + S], FP16)
        seg_sb = segio_sb[:, 0:T]
        iota_sb = segio_sb[:, T : T + S]
        # separate copy for ACT to avoid SBUF bank contention with DVE reads
        iota2_sb = sb.tile([P, S], FP16)
        feat_sb = sb.tile([P, T * F], FP16)
        combw6_sb = sb.tile([P, CW], FP16)
        nc.sync.dma_start(out=segio_sb[:], in_=segio.ap()[:, :])
        nc.scalar.dma_start(out=iota2_sb[:], in_=iota_c.ap()[:, :])
        for q in range(4):
            qs = q * (T * F // 4)
            qe = (q + 1) * (T * F // 4)
            nc.sync.dma_start(out=feat_sb[:, qs:qe], in_=feat.ap()[:, qs:qe])
        nc.gpsimd.dma_start(out=combw6_sb[:], in_=combw6.ap()[:, :])

        # fp32 copy of seg for the is_equal scalar operand
        seg32_sb = sb.tile([P, T], F32)
        nc.vector.tensor_copy(seg32_sb[:], seg_sb)
        if use_act:
            nseg4_sb = sb.tile([P, T], F32)
            nc.scalar.activation(
                nseg4_sb[:],
                seg_sb,
                mybir.ActivationFunctionType.Copy,
                bias=0.0,
                scale=-4.0,
            )

        # chunk -> engine, chunk -> PSUM column-group (act has its own group).
        # The last chunks go to the faster V queue so the final matmuls are
        # not head-of-line blocked behind a slow ACT one-hot.
        engs = [pattern[t % len(pattern)] for t in range(T)]
        groups = []
        rr = 0
        for eng in engs:
            if eng == "act" and act_group is not None:
                groups.append(act_group)
            else:
                groups.append(rot_groups[rr % len(rot_groups)])
                rr += 1
        first_t = {}
        last_t = {}
        for t, j in enumerate(groups):
            first_t.setdefault(j, t)
            last_t[j] = t

        # phase 1: segment sums accumulated in PSUM
        acc = pp.tile([P, 512], F32)  # full bank so PSUM tiles stay bank-aligned
        # zero the rows the col-tiled matmuls never touch (comb does 0*garbage
        # otherwise, and PSUM garbage can be NaN)
        nc.vector.memset(acc[:], 0.0)
        for t in range(T):
            j = groups[t]
            eng = engs[t]
            oh = ohp.tile([P, S], FP16, tag=f"oh_{eng}")
            if eng == "dve":
                nc.vector.tensor_scalar(
                    out=oh[:],
                    in0=iota_sb,
                    scalar1=seg32_sb[:, t : t + 1],
                    scalar2=None,
                    op0=mybir.AluOpType.is_equal,
                )
            elif eng == "gpsimd":
                nc.gpsimd.tensor_scalar(
                    out=oh[:],
                    in0=iota_sb,
                    scalar1=seg32_sb[:, t : t + 1],
                    scalar2=None,
                    op0=mybir.AluOpType.is_equal,
                )
            elif eng == "act":
                # erf'(4*(iota-seg)) = (2/sqrt(pi)) * exact one-hot in fp16
                nc.scalar.activation(
                    oh[:],
                    iota2_sb[:],
                    mybir.ActivationFunctionType.Derivative_Erf,
                    bias=nseg4_sb[:, t : t + 1],
                    scale=4.0,
                )
            else:
                raise ValueError(eng)
            nc.tensor.matmul(
                acc[32 * j : 32 * j + F, 0:S],
                lhsT=feat_sb[:, F * t : F * (t + 1)],
                rhs=oh[:],
                start=(t == first_t[j]),
                stop=(t == last_t[j]),
                tile_position=(0, 32 * j) if col_tile else None,
                skip_group_check=True,
            )

        # tail: out_raw[s, 0:768] = acc.T @ combw6 gives the projected segment
        # sums, col 768 the counts (segments now sit on PSUM partitions, so
        # the reciprocal is a per-partition FD=1 op); then scale each row.
        acc_sb = sb.tile([P, S], FP16)
        with nc.allow_low_precision(reason="bf16 tail; tolerance is 2e-2"):
            nc.vector.tensor_copy(acc_sb[:, 0:P], acc[:, 0:P])
            nc.vector.tensor_copy(acc_sb[:, P:S], acc[:, P:S])
        for lo, hi in ((0, P), (P, S)):
            m = hi - lo
            otile = pp.tile([P, 1024], F32, tag=f"ot{lo}")
            nc.tensor.matmul(
                otile[:m, 512:CW],
                lhsT=acc_sb[:, lo:hi],
                rhs=combw6_sb[:, 512:CW],
                start=True,
                stop=True,
            )
            nc.tensor.matmul(
                otile[:m, 0:512],
                lhsT=acc_sb[:, lo:hi],
                rhs=combw6_sb[:, 0:512],
                start=True,
                stop=True,
            )
            cnt_sb = sb.tile([P, 1], F32, tag=f"cnt{lo}")
            nc.vector.tensor_scalar(
                out=cnt_sb[:m],
                in0=otile[:m, E : E + 1],
                scalar1=1.0,
                scalar2=None,
                op0=mybir.AluOpType.max,
            )
            rec_sb = sb.tile([P, 1], F32, tag=f"rec{lo}")
            nc.vector.reciprocal(rec_sb[:m], cnt_sb[:m])
            # the two row-tiles' scale-by-recip run concurrently: ACT (idle
            # by now) takes the first, V the second
            ob = sb.tile([P, E], FP16, tag=f"ob{lo}")
            if lo == 0:
                nc.scalar.activation(
                    ob[:m],
                    otile[:m, 0:E],
                    mybir.ActivationFunctionType.Copy,
                    scale=rec_sb[:m, 0:1],
                )
            else:
                with nc.allow_low_precision(reason="bf16 out; 2e-2 tol"):
                    nc.vector.tensor_scalar(
                        out=ob[:m],
                        in0=otile[:m, 0:E],
                        scalar1=rec_sb[:m, 0:1],
                        scalar2=None,
                        op0=mybir.AluOpType.mult,
                    )
            # separate queues: DMA completions serialize ~2.5us per queue
            eng = nc.sync if lo == 0 else nc.scalar
            eng.dma_start(out=out.ap()[lo:hi, :], in_=ob[:m, :])

    return nc




_PROGRAM_CACHE = {}
_PATTERN = ("dve", "dve", "act")


def run(inputs, trace=False, pattern=_PATTERN, oh_bufs=12, **bkw):
    from concourse.bass_utils import run_bass_kernel_spmd

    img = np.asarray(inputs["img"]).astype(np.float32)
    segments = np.asarray(inputs["segments"])
    W = np.asarray(inputs["W"]).astype(np.float32)
    b = np.asarray(inputs["b"]).astype(np.float32)

    in_maps = _prep_core_inputs(img, segments, W, b)
    nc = _build_program(pattern=pattern, oh_bufs=oh_bufs)
    res = run_bass_kernel_spmd(nc, in_maps, list(range(B)), trace=trace, **bkw)
    out = np.stack(
        [np.asarray(res.results[i]["out"]) for i in range(B)]
    ).astype(np.float32)
    return out, res


def kernel(**inputs) -> np.ndarray:
    out, _ = run(inputs)
    return out

